# revision 8
# baseline (speedup 1.0000x reference)
"""Trainium2 Bass kernel for nn_LstmCrfModel (embedding -> LSTM -> dense -> CRF ll).

Sharding: pure data parallel, batch 64 split as 8 sequences per NeuronCore.

Per-core design (B=8 local, T=512, E=128, H=512, L=9):
  - lens   = sum(text != 0) per row.
  - x.T    : embedding rows gathered 128 tokens at a time (indirect DMA) in
             T-major token order (col = t*8+b), PE-transposed into [E, 4096].
  - LSTM   : 512 fully unrolled steps. Gates kept in [batch, gate] layout:
             each gate's [8,512] PSUM accumulates  x_t.T-stationary @ Wx-chunk
             plus 4x  h.T-stationary @ Wh-chunk  matmuls (moving operand = the
             weight matrix, stationary = skinny activation columns).
             Keras bias handled as per-gate scalar constants in the ACT bias.
             h is PE-transposed each step into an [128, 4, 4096] h.T history
             (serves as next-step stationary and the dense-layer operand).
  - dense  : per-step [9,8] logits PSUM from 4 Wd-chunk matmuls; stored to a
             [9, 4096] logits.T buffer (host transposes back).
  - CRF    : exp-domain forward scan fused into the step loop:
             a <- (exp(trans).T @ a) * exp(logit_t), with per-batch-column
             freeze (copy_predicated on t < lens) and periodic renormalization
             every 8 steps (transpose dance; log-scales accumulated in logC).
             score (unary+binary) computed via one-hot/mask algebra at the end.
Outputs per core: logitsT [9, 4096] f32, lens [8,1] i32, ll [8,1] f32.
"""
import numpy as np

import concourse.bass as bass
import concourse.tile as tile
from concourse import mybir
from concourse.bass import IndirectOffsetOnAxis
from concourse.bass_utils import run_bass_kernel_spmd
from concourse.masks import make_identity

f32 = mybir.dt.float32
i32 = mybir.dt.int32
bf16 = mybir.dt.bfloat16
AF = mybir.ActivationFunctionType
ALU = mybir.AluOpType

V, E, H, L = 100000, 128, 512, 9
B, T = 64, 512
NC = 8
BL = B // NC  # 8 sequences per core
KH = H // 128  # 4 k-chunks of the hidden dim
G4 = 4 * H  # 2048
RENORM = 8

MAXW = 1  # walrus in this container: one sync wait per instruction


def split_sync_waits(nc, maxw=MAXW):
    """Move excess per-instruction sync waits onto preceding NoOps (same engine)."""
    nid = 0
    for f in nc.m.functions:
        for blk in f.blocks:
            out = []
            changed = False
            for inst in blk.instructions:
                si = getattr(inst, "sync_info", None)
                if si is not None:
                    waits = list(si.on_wait)
                    if len(waits) > maxw:
                        changed = True
                        keep = waits[-maxw:]
                        excess = waits[:-maxw]
                        inst.sync_info = mybir.SyncInfo(
                            on_wait=keep, on_update=list(si.on_update))
                        for i in range(0, len(excess), maxw):
                            nid += 1
                            out.append(mybir.InstNoOp(
                                name=f"waitsplit-{nid}",
                                engine=inst.engine,
                                ins=[], outs=[],
                                sync_info=mybir.SyncInfo(
                                    on_wait=excess[i:i + maxw], on_update=[])))
                out.append(inst)
            if changed:
                blk.instructions = out


def build_module(TT=T, gate_bias=(0.0, 1.0, 0.0, 0.0), general_bias=False):
    """gate order in the 4H axis is Keras: i, f, cg, o."""
    nc = bass.Bass("TRN2", target_bir_lowering=False, debug=False)
    NTOK = TT * BL
    NCHUNK = NTOK // 128

    d_text = nc.dram_tensor("text", [BL, TT], i32, kind="ExternalInput")
    d_textT = nc.dram_tensor("textT", [NTOK], i32, kind="ExternalInput")
    d_labT = nc.dram_tensor("labelsT", [NTOK], i32, kind="ExternalInput")
    d_embed = nc.dram_tensor("embed", [V, E], f32, kind="ExternalInput")
    d_Wx = nc.dram_tensor("Wx", [E, G4], f32, kind="ExternalInput")
    d_Wh = nc.dram_tensor("Wh", [H, G4], f32, kind="ExternalInput")
    d_Wd = nc.dram_tensor("Wd", [H, L], f32, kind="ExternalInput")
    d_bd = nc.dram_tensor("bd", [L], f32, kind="ExternalInput")
    d_trans = nc.dram_tensor("trans", [L, L], f32, kind="ExternalInput")
    d_bfull = nc.dram_tensor("bfull", [G4], f32, kind="ExternalInput")

    d_logitsT = nc.dram_tensor("logitsT", [L, NTOK], f32, kind="ExternalOutput")
    d_lens = nc.dram_tensor("lens", [BL, 1], i32, kind="ExternalOutput")
    d_ll = nc.dram_tensor("ll", [BL, 1], f32, kind="ExternalOutput")

    gate_act = [AF.Sigmoid, AF.Sigmoid, AF.Tanh, AF.Sigmoid]  # i, f, cg, o

    with tile.TileContext(nc) as tc:
        with (
            tc.tile_pool(name="persist", bufs=1) as P1,
            tc.tile_pool(name="step", bufs=2) as SP,
            tc.tile_pool(name="scratch", bufs=1) as SC,
            tc.tile_pool(name="psum", bufs=1, space="PSUM") as PS,
        ):
            ident = P1.tile([128, 128], f32)
            make_identity(nc, ident[:])

            # ---------------- lens ----------------
            text_sb = P1.tile([BL, TT], i32)
            nc.sync.dma_start(out=text_sb[:], in_=d_text[:, :])
            nz = SC.tile([BL, TT], f32, tag="sY", name="nz")
            nc.vector.tensor_scalar(out=nz[:], in0=text_sb[:], scalar1=0,
                                    scalar2=None, op0=ALU.not_equal)
            lens_f = P1.tile([BL, 1], f32)
            nc.vector.tensor_reduce(out=lens_f[:], in_=nz[:],
                                    axis=mybir.AxisListType.X, op=ALU.add)
            lens_i = P1.tile([BL, 1], i32)
            nc.vector.tensor_copy(out=lens_i[:], in_=lens_f[:])
            nc.sync.dma_start(out=d_lens[:, :], in_=lens_i[:])

            # ---------------- weights ----------------
            Wx_sb = P1.tile([E, G4], f32)
            nc.sync.dma_start(out=Wx_sb[:], in_=d_Wx[:, :])
            Wh_sb = P1.tile([128, KH, G4], f32)
            nc.sync.dma_start(out=Wh_sb[:],
                              in_=d_Wh[:, :].rearrange("(k p) g -> p k g", p=128))
            Wd_sb = P1.tile([128, KH, L], f32)
            nc.sync.dma_start(out=Wd_sb[:],
                              in_=d_Wd[:, :].rearrange("(k p) l -> p k l", p=128))
            bd_sb = P1.tile([L, 1], f32)
            nc.sync.dma_start(out=bd_sb[:], in_=d_bd[:, None])
            trans_sb = P1.tile([L, L], f32)
            nc.sync.dma_start(out=trans_sb[:], in_=d_trans[:, :])
            E_sb = P1.tile([L, L], f32)
            nc.scalar.activation(out=E_sb[:], in_=trans_sb[:], func=AF.Exp)
            bias_sb = None
            if general_bias:
                bias_sb = P1.tile([BL, G4], f32)
                nc.sync.dma_start(out=bias_sb[:],
                                  in_=d_bfull[None, :].to_broadcast((BL, G4)))

            # ---------------- embedding gather + transpose ----------------
            textT_sb = P1.tile([128, NCHUNK], i32)
            nc.sync.dma_start(out=textT_sb[:],
                              in_=d_textT[:].rearrange("(c p) -> p c", p=128))
            xT_sb = P1.tile([E, NTOK], f32)
            for c in range(NCHUNK):
                x_sb = SP.tile([128, E], f32, tag="gath")
                nc.gpsimd.indirect_dma_start(
                    out=x_sb[:], out_offset=None, in_=d_embed[:, :],
                    in_offset=IndirectOffsetOnAxis(ap=textT_sb[:, c:c + 1], axis=0),
                )
                xT_ps = PS.tile([128, 128], f32, space="PSUM", tag=f"pg{c % 2}",
                                name="xT_ps")
                nc.tensor.transpose(out=xT_ps[:], in_=x_sb[:], identity=ident[:])
                nc.vector.tensor_copy(out=xT_sb[:, c * 128:(c + 1) * 128], in_=xT_ps[:])

            # ---------------- step masks (int, for copy_predicated) ----------------
            BCH = [(s, min(s + 512, NTOK)) for s in range(0, NTOK, 512)]
            ones1_9 = P1.tile([1, L], f32)
            nc.vector.memset(ones1_9[:], 1.0)
            ones9 = P1.tile([L, 1], f32)
            nc.vector.memset(ones9[:], 1.0)
            iota_t = SC.tile([1, NTOK], i32, tag="sX", name="iota_t")
            nc.gpsimd.iota(iota_t[:], pattern=[[1, TT], [0, BL]], base=0,
                           channel_multiplier=0)
            iota_tf = SC.tile([1, NTOK], f32, tag="sY", name="iota_tf")
            nc.vector.tensor_copy(out=iota_tf[:], in_=iota_t[:])
            lensT_ps = PS.tile([1, BL], f32, space="PSUM", tag="misc")
            nc.tensor.transpose(out=lensT_ps[:], in_=lens_f[:], identity=ident[:BL, :BL])
            lensT_sb = P1.tile([1, BL], f32)
            nc.vector.tensor_copy(out=lensT_sb[:], in_=lensT_ps[:])
            mask1 = SC.tile([1, NTOK], f32, tag="sO", name="mask1")
            nc.vector.tensor_tensor(
                out=mask1[:].rearrange("p (t b) -> p t b", b=BL),
                in0=iota_tf[:].rearrange("p (t b) -> p t b", b=BL),
                in1=lensT_sb[:1, None, :].to_broadcast((1, TT, BL)),
                op=ALU.is_lt)
            masks9i = P1.tile([L, NTOK], i32)
            for n, (s0, s1) in enumerate(BCH):
                mp = PS.tile([L, 512], f32, space="PSUM", tag=f"pg{2 + n % 2}")
                nc.tensor.matmul(mp[:, :s1 - s0], ones1_9[:], mask1[:, s0:s1],
                                 start=True, stop=True)
                nc.vector.tensor_copy(out=masks9i[:, s0:s1], in_=mp[:, :s1 - s0])

            # ---------------- LSTM + dense + CRF scan ----------------
            c_sb = P1.tile([BL, H], f32)
            nc.vector.memset(c_sb[:], 0.0)
            hT_hist = P1.tile([128, KH, 2, BL], f32)
            logitsT = P1.tile([L, NTOK], f32)
            a_sb = P1.tile([L, BL], f32)
            logC = P1.tile([BL, 1], f32)
            nc.vector.memset(logC[:], 0.0)

            GORDER = [1, 2, 0, 3]  # f, cg, i, o
            pg = {}
            for t in range(TT):
                ts8 = slice(t * BL, (t + 1) * BL)
                # gate matmuls
                for g in GORDER:
                    pg[g] = PS.tile([BL, 512], f32, space="PSUM", tag=f"pg{g}",
                                    name=f"pg{g}")
                    gs = slice(g * 512, (g + 1) * 512)
                    nc.tensor.matmul(pg[g][:], xT_sb[:, ts8], Wx_sb[:, gs],
                                     start=True, stop=(t == 0))
                    if t > 0:
                        for k in range(KH):
                            nc.tensor.matmul(pg[g][:],
                                             hT_hist[:, k, (t - 1) % 2, :],
                                             Wh_sb[:, k, gs],
                                             start=False, stop=(k == KH - 1))
                # gate activations
                act = {}
                for g in GORDER:
                    gin = pg[g][:]
                    if general_bias:
                        gb = SP.tile([BL, 512], f32, tag=f"gb{g}")
                        nc.vector.tensor_add(out=gb[:], in0=gin,
                                             in1=bias_sb[:, g * 512:(g + 1) * 512])
                        gin = gb[:]
                        bias_c = 0.0
                    else:
                        bias_c = float(gate_bias[g])
                    act[g] = SP.tile([BL, 512], f32, tag=f"act{g}", name=f"act{g}")
                    nc.scalar.activation(out=act[g][:], in_=gin,
                                         func=gate_act[g], bias=bias_c)
                # c update
                m1 = SP.tile([BL, H], f32, tag="m1")
                nc.vector.tensor_mul(out=m1[:], in0=act[1][:], in1=c_sb[:])
                m2 = SP.tile([BL, H], f32, tag="m2")
                nc.vector.tensor_mul(out=m2[:], in0=act[0][:], in1=act[2][:])
                nc.vector.tensor_add(out=c_sb[:], in0=m1[:], in1=m2[:])
                th = SP.tile([BL, H], f32, tag="th")
                nc.scalar.activation(out=th[:], in_=c_sb[:], func=AF.Tanh)
                h_sb = SP.tile([BL, H], f32, tag="h")
                nc.vector.tensor_mul(out=h_sb[:], in0=act[3][:], in1=th[:])
                # transpose h into history
                ph = PS.tile([128, KH * BL], f32, space="PSUM", tag="ph")
                for k in range(KH):
                    nc.tensor.transpose(out=ph[:, k * BL:(k + 1) * BL],
                                        in_=h_sb[:, k * 128:(k + 1) * 128],
                                        identity=ident[:BL, :BL])
                nc.vector.tensor_copy(
                    out=hT_hist[:, :, t % 2, :],
                    in_=ph[:].rearrange("p (k b) -> p k b", b=BL))
                # logits
                pl = PS.tile([L, BL], f32, space="PSUM", tag="pl")
                for k in range(KH):
                    nc.tensor.matmul(pl[:], Wd_sb[:, k, :], hT_hist[:, k, t % 2, :],
                                     start=(k == 0), stop=(k == KH - 1))
                nc.scalar.activation(out=logitsT[:, ts8], in_=pl[:],
                                     func=AF.Identity, bias=bd_sb[:, :1])
                # CRF
                if t == 0:
                    nc.scalar.activation(out=a_sb[:], in_=pl[:], func=AF.Exp,
                                         bias=bd_sb[:, :1])
                else:
                    elg = SP.tile([L, BL], f32, tag="elg")
                    nc.scalar.activation(out=elg[:], in_=pl[:], func=AF.Exp,
                                         bias=bd_sb[:, :1])
                    pa = PS.tile([L, BL], f32, space="PSUM", tag="pa")
                    nc.tensor.matmul(pa[:], E_sb[:], a_sb[:], start=True, stop=True)
                    anew = SP.tile([L, BL], f32, tag="anew")
                    nc.vector.tensor_mul(out=anew[:], in0=pa[:], in1=elg[:])
                    nc.vector.copy_predicated(out=a_sb[:], mask=masks9i[:, ts8],
                                              data=anew[:])
                    if t % RENORM == RENORM - 1:
                        paT = PS.tile([BL, L], f32, space="PSUM", tag="misc")
                        nc.tensor.transpose(out=paT[:], in_=a_sb[:],
                                            identity=ident[:L, :L])
                        aT = SP.tile([BL, L], f32, tag="aT")
                        nc.vector.tensor_copy(out=aT[:], in_=paT[:])
                        s8 = SP.tile([BL, 1], f32, tag="s8")
                        nc.vector.tensor_reduce(out=s8[:], in_=aT[:],
                                                axis=mybir.AxisListType.X, op=ALU.add)
                        r8 = SP.tile([BL, 1], f32, tag="r8")
                        nc.vector.reciprocal(out=r8[:], in_=s8[:])
                        nc.vector.tensor_scalar_mul(out=aT[:], in0=aT[:], scalar1=r8[:])
                        l8 = SP.tile([BL, 1], f32, tag="l8")
                        nc.scalar.activation(out=l8[:], in_=s8[:], func=AF.Ln)
                        nc.vector.tensor_add(out=logC[:], in0=logC[:], in1=l8[:])
                        paB = PS.tile([L, BL], f32, space="PSUM", tag="pa")
                        nc.tensor.transpose(out=paB[:], in_=aT[:],
                                            identity=ident[:BL, :BL])
                        nc.vector.tensor_copy(out=a_sb[:], in_=paB[:])

            nc.sync.dma_start(out=d_logitsT[:, :], in_=logitsT[:])

            # ---------------- log_norm ----------------
            paT = PS.tile([BL, L], f32, space="PSUM", tag="misc")
            nc.tensor.transpose(out=paT[:], in_=a_sb[:], identity=ident[:L, :L])
            aT = SP.tile([BL, L], f32, tag="aT")
            nc.vector.tensor_copy(out=aT[:], in_=paT[:])
            sfin = P1.tile([BL, 1], f32)
            nc.vector.tensor_reduce(out=sfin[:], in_=aT[:],
                                    axis=mybir.AxisListType.X, op=ALU.add)
            lognorm = P1.tile([BL, 1], f32)
            nc.scalar.activation(out=lognorm[:], in_=sfin[:], func=AF.Ln)
            nc.vector.tensor_add(out=lognorm[:], in0=lognorm[:], in1=logC[:])

            # ---------------- score ----------------
            # one-hot of labels, pre-masked by (t < lens); the binary B-side
            # mask (t+1 < lens) implies the A-side mask so one masked one-hot
            # serves unary and both binary factors.
            labT_sb = SC.tile([1, NTOK], i32, tag="sX", name="labT_sb")
            nc.sync.dma_start(out=labT_sb[:], in_=d_labT[None, :])
            labT_f = SC.tile([1, NTOK], f32, tag="sY", name="labT_f")
            nc.vector.tensor_copy(out=labT_f[:], in_=labT_sb[:])
            onehot9 = SC.tile([L, NTOK], f32, tag="sO", name="onehot9")
            for n, (s0, s1) in enumerate(BCH):
                lp = PS.tile([L, 512], f32, space="PSUM", tag=f"pg{2 + n % 2}")
                nc.tensor.matmul(lp[:, :s1 - s0], ones1_9[:], labT_f[:, s0:s1],
                                 start=True, stop=True)
                nc.vector.tensor_copy(out=onehot9[:, s0:s1], in_=lp[:, :s1 - s0])
            iota9 = SC.tile([L, NTOK], i32, tag="sX", name="iota9")
            nc.gpsimd.iota(iota9[:], pattern=[[0, NTOK]], base=0, channel_multiplier=1)
            iota9f = SC.tile([L, NTOK], f32, tag="sY", name="iota9f")
            nc.vector.tensor_copy(out=iota9f[:], in_=iota9[:])
            nc.vector.tensor_tensor(out=onehot9[:], in0=onehot9[:], in1=iota9f[:],
                                    op=ALU.is_equal)
            masks9f = SC.tile([L, NTOK], f32, tag="sX", name="masks9f")
            nc.vector.tensor_copy(out=masks9f[:], in_=masks9i[:])
            nc.vector.tensor_mul(out=onehot9[:], in0=onehot9[:], in1=masks9f[:])
            # unary
            u9 = SC.tile([L, NTOK], f32, tag="sY", name="u9")
            nc.vector.tensor_mul(out=u9[:], in0=logitsT[:], in1=onehot9[:])
            u9r = P1.tile([L, BL], f32)
            nc.vector.tensor_reduce(out=u9r[:],
                                    in_=u9[:].rearrange("l (t b) -> l b t", b=BL),
                                    axis=mybir.AxisListType.X, op=ALU.add)
            sc_ps = PS.tile([1, BL], f32, space="PSUM", tag="pl")
            nc.tensor.matmul(sc_ps[:], ones9[:], u9r[:], start=True, stop=False)
            # binary: C9[j, tb] = trans[lab(tb), j]; BB = C9[:, :-8] * onehot9[:, 8:]
            c9 = SC.tile([L, NTOK], f32, tag="sX", name="c9")
            for n, (s0, s1) in enumerate(BCH):
                cp = PS.tile([L, 512], f32, space="PSUM", tag=f"pg{2 + n % 2}")
                nc.tensor.matmul(cp[:, :s1 - s0], trans_sb[:], onehot9[:, s0:s1],
                                 start=True, stop=True)
                nc.vector.tensor_copy(out=c9[:, s0:s1], in_=cp[:, :s1 - s0])
            nw = NTOK - BL
            bb = SC.tile([L, nw], f32, tag="sY", name="bb")
            nc.vector.tensor_mul(out=bb[:], in0=c9[:, :nw], in1=onehot9[:, BL:])
            b9r = P1.tile([L, BL], f32)
            nc.vector.tensor_reduce(out=b9r[:],
                                    in_=bb[:].rearrange("l (t b) -> l b t", b=BL),
                                    axis=mybir.AxisListType.X, op=ALU.add)
            nc.tensor.matmul(sc_ps[:], ones9[:], b9r[:], start=False, stop=True)
            sc_sb = P1.tile([1, BL], f32)
            nc.vector.tensor_copy(out=sc_sb[:], in_=sc_ps[:])
            scT_ps = PS.tile([BL, 1], f32, space="PSUM", tag="misc")
            nc.tensor.transpose(out=scT_ps[:], in_=sc_sb[:], identity=ident[:1, :1])
            score8 = P1.tile([BL, 1], f32)
            nc.vector.tensor_copy(out=score8[:], in_=scT_ps[:])
            ll8 = P1.tile([BL, 1], f32)
            nc.vector.tensor_sub(out=ll8[:], in0=score8[:], in1=lognorm[:])
            nc.sync.dma_start(out=d_ll[:, :], in_=ll8[:])

    split_sync_waits(nc)
    return nc


_module_cache = {}


def _get_module(TT, gate_bias, general_bias):
    key = (TT, gate_bias, general_bias)
    if key not in _module_cache:
        _module_cache[key] = build_module(TT, gate_bias, general_bias)
    return _module_cache[key]


def _gate_bias_of(b):
    gates = b.reshape(4, H)
    if all(np.all(g == g[0]) for g in gates):
        return tuple(float(g[0]) for g in gates), False
    return (0.0, 0.0, 0.0, 0.0), True


def kernel(text, labels, embed, Wx, Wh, b, Wd, bd, trans, TT=T, trace=False):
    text = np.asarray(text)
    labels = np.asarray(labels)
    embed = np.ascontiguousarray(np.asarray(embed), dtype=np.float32)
    Wx = np.ascontiguousarray(np.asarray(Wx), dtype=np.float32)
    Wh = np.ascontiguousarray(np.asarray(Wh), dtype=np.float32)
    b = np.asarray(b, dtype=np.float32)
    Wd = np.ascontiguousarray(np.asarray(Wd), dtype=np.float32)
    bd = np.ascontiguousarray(np.asarray(bd), dtype=np.float32)
    trans = np.ascontiguousarray(np.asarray(trans), dtype=np.float32)

    gate_bias, general_bias = _gate_bias_of(b)
    nc = _get_module(TT, gate_bias, general_bias)

    in_maps = []
    for k in range(NC):
        sl = slice(k * BL, (k + 1) * BL)
        tloc = np.ascontiguousarray(text[sl, :TT], dtype=np.int32)
        lloc = np.ascontiguousarray(labels[sl, :TT], dtype=np.int32)
        in_maps.append(dict(
            text=tloc,
            textT=np.ascontiguousarray(tloc.T.reshape(-1)),
            labelsT=np.ascontiguousarray(lloc.T.reshape(-1)),
            embed=embed, Wx=Wx, Wh=Wh, Wd=Wd, bd=bd, trans=trans,
            bfull=b,
        ))

    res = run_bass_kernel_spmd(nc, in_maps, core_ids=list(range(NC)), trace=trace)

    logits = np.empty((B, TT, L), dtype=np.float32)
    lens = np.empty((B,), dtype=np.int32)
    ll = np.empty((B,), dtype=np.float32)
    for k, r in enumerate(res.results):
        sl = slice(k * BL, (k + 1) * BL)
        logits[sl] = r["logitsT"].reshape(L, TT, BL).transpose(2, 1, 0)
        lens[sl] = r["lens"][:, 0]
        ll[sl] = r["ll"][:, 0]
    kernel.last_result = res
    return logits, lens, ll
